# revision 18
# baseline (speedup 1.0000x reference)
"""GCN layer (copy_u -> mean -> linear) on 8 Trainium2 NeuronCores.

Sharding: nodes partitioned across the 8 cores by dst range (6250 each).
Each core aggregates messages for its dst range, pulling src features from
a full replica of the feature table in its HBM via dma_gather (1024-index
calls — SWDGE ring limit), reduces per-block segments with one-hot matmuls
in PSUM, scatter-adds unique per-block partials into an HBM accumulator
(8 blocks batched per call, stride-grouped so no real row repeats within a
call), then applies mean/blend and the linear layer.
"""

import os
import time

import numpy as np

from concourse import bacc, mybir, tile
from concourse.library_config import mlp

N_NODES = 50000
DIM = 256
NCORES = 8
NSH = N_NODES // NCORES          # 6250 dst nodes per core
SPLIT = 32768                    # int16 index limit for dma_gather
ABLK = 2048                      # slots per logical A-block (src < SPLIT)
BBLK = 1024                      # slots per logical B-block (src >= SPLIT)
GCALL = 1024                     # max indices per dma_gather call (ring limit)
GRP = 8                          # logical blocks per scatter call
MAXSEG = 128                     # max distinct dsts per block (PSUM partitions)
RANK_PAD = 300.0                 # rank sentinel for dummy slots
MSUM_ROWS = 6400                 # 6250 real + dummy rows 6250..6399
OUT_ROWS = 6272                  # 49 * 128
NT = OUT_ROWS // 128             # row tiles in phase 2

_last_exec_ns = None
_last_times = None


def _blocks_for_bucket(e_src, e_dst, blk):
    """Split sorted-by-dst edges into blocks of <=blk slots and <=MAXSEG
    distinct dsts. Returns list of (src16, rank_f32, seg_dst16, lo, hi)."""
    n = e_src.shape[0]
    blocks = []
    if n == 0:
        return blocks
    change = np.empty(n, np.int64)
    change[0] = 0
    change[1:] = (e_dst[1:] != e_dst[:-1]).astype(np.int64)
    runid = np.cumsum(change)
    start = 0
    while start < n:
        end = min(start + blk, n)
        lim = np.searchsorted(runid, runid[start] + MAXSEG, side="left")
        end = min(end, lim)
        assert end > start
        cnt = end - start
        src16 = np.zeros(blk, np.int16)
        src16[:cnt] = e_src[start:end].astype(np.int16)
        rank = np.full(blk, RANK_PAD, np.float32)
        rank[:cnt] = (runid[start:end] - runid[start]).astype(np.float32)
        nseg = int(runid[end - 1] - runid[start]) + 1
        seg_dst = (np.arange(MAXSEG, dtype=np.int64) % 150) + NSH  # dummies
        firsts = start + np.flatnonzero(np.r_[1, change[start + 1:end]])
        seg_dst[:nseg] = e_dst[firsts]
        blocks.append((src16, rank, seg_dst.astype(np.int16),
                       int(e_dst[start]), int(e_dst[end - 1])))
        start = end
    return blocks


def _dummy_block(blk):
    return (np.zeros(blk, np.int16),
            np.full(blk, RANK_PAD, np.float32),
            ((np.arange(MAXSEG) % 150) + NSH).astype(np.int16),
            -1, -1)


def _wrap16(x):
    """index layout for one SWDGE call: idx j -> partition j%16, col j//16,
    replicated 8x across the 128 partitions."""
    w = x.reshape(-1, 16).T.astype(np.int16)
    return np.tile(w, (8, 1))


def _wrap_calls(x, call):
    """wrap an index array chunked into `call`-sized SWDGE calls."""
    return np.concatenate(
        [_wrap16(x[i:i + call]) for i in range(0, x.size, call)], axis=1)


def _slotmajor(x, blk):
    """slot j -> partition j%128, col j//128."""
    return x.reshape(blk // 128, 128).T.copy()


def _grp_for(nblk):
    """Scatter batching factor: stride nblk//grp must be >= 2 so blocks in
    one scatter call never share a dst row."""
    return max(1, min(GRP, nblk // 2))


def _emit_order(nblk, grp):
    """Permute block indices so each consecutive `grp` in emission order are
    nblk//grp apart in dst order (no shared dst rows within a scatter call)."""
    g = nblk // grp
    order = []
    for i in range(g):
        for k in range(grp):
            order.append(i + k * g)
    assert sorted(order) == list(range(nblk))
    return order


def _build_core_inputs(c, feature, src, dst, W, b):
    sel = (dst >= c * NSH) & (dst < (c + 1) * NSH)
    es = src[sel]
    ed = dst[sel] - c * NSH
    order = np.argsort(ed, kind="stable")
    es, ed = es[order], ed[order]

    deg = np.bincount(ed, minlength=NSH).astype(np.float32)
    alpha = np.zeros(OUT_ROWS, np.float32)
    beta = np.zeros(OUT_ROWS, np.float32)
    alpha[:NSH] = np.where(deg > 0, 1.0 / np.maximum(deg, 1.0), 0.0)
    beta[:NSH] = (deg == 0).astype(np.float32)

    lo = es < SPLIT
    blocksA = _blocks_for_bucket(es[lo], ed[lo], ABLK)
    blocksB = _blocks_for_bucket(es[~lo] - SPLIT, ed[~lo], BBLK)

    featsh = np.zeros((OUT_ROWS, DIM), np.float32)
    featsh[:NSH] = feature[c * NSH:(c + 1) * NSH]

    return {
        "blocksA": blocksA, "blocksB": blocksB,
        "alpha": alpha, "beta": beta, "featsh": featsh,
    }


def _pack_bucket(blocks, nblk, grp, blk):
    """Pad to nblk blocks, permute into emission order, build arrays."""
    blocks = list(blocks)
    while len(blocks) < nblk:
        blocks.append(_dummy_block(blk))
    order = _emit_order(nblk, grp)
    blocks = [blocks[i] for i in order]
    # verify no duplicate real dst rows within any scatter group
    for g0 in range(0, nblk, grp):
        real = np.concatenate([blocks[i][2] for i in range(g0, g0 + grp)])
        real = real[real < NSH]
        assert real.size == np.unique(real).size, "dup dst in scatter group"
    si = np.concatenate([_wrap_calls(x[0], GCALL) for x in blocks], axis=1)
    rk = np.concatenate([_slotmajor(x[1], blk) for x in blocks], axis=1)
    dsc = np.concatenate(
        [_wrap16(np.concatenate([blocks[g0 + k][2] for k in range(grp)]))
         for g0 in range(0, nblk, grp)], axis=1)
    return si, rk, dsc


def _finalize_core_inputs(core, nblkA, nblkB, grpA, grpB, feature, W, b):
    siA, rkA, dscA = _pack_bucket(core["blocksA"], nblkA, grpA, ABLK)
    siB, rkB, dscB = _pack_bucket(core["blocksB"], nblkB, grpB, BBLK)
    return {
        "feature": np.ascontiguousarray(feature),
        "siA": np.ascontiguousarray(siA), "siB": np.ascontiguousarray(siB),
        "rkA": np.ascontiguousarray(rkA), "rkB": np.ascontiguousarray(rkB),
        "dsc": np.ascontiguousarray(np.concatenate([dscA, dscB], axis=1)),
        "alphaT": np.ascontiguousarray(core["alpha"].reshape(NT, 128).T),
        "betaT": np.ascontiguousarray(core["beta"].reshape(NT, 128).T),
        "featsh": core["featsh"],
        "WT": np.ascontiguousarray(W.T.reshape(2, 128, DIM)),
        "bias": np.ascontiguousarray(b.reshape(1, DIM)),
        "ident": np.eye(128, dtype=np.float32),
        "iota": np.tile(np.arange(128, dtype=np.float32), (128, ABLK // 128)),
    }


def _round_up(x, m):
    return (x + m - 1) // m * m


def _build_program(nblkA, nblkB, grpA, grpB):
    f32, i16 = mybir.dt.float32, mybir.dt.int16
    nc = bacc.Bacc("TRN2")

    ngrpA, ngrpB = nblkA // grpA, nblkB // grpB
    ndsc16 = (ngrpA * grpA + ngrpB * grpB) * (MAXSEG // 16)
    feature = nc.declare_dram_parameter("feature", [N_NODES, DIM], f32, isOutput=False)
    siA = nc.declare_dram_parameter("siA", [128, nblkA * (ABLK // 16)], i16, isOutput=False)
    siB = nc.declare_dram_parameter("siB", [128, nblkB * (BBLK // 16)], i16, isOutput=False)
    rkA = nc.declare_dram_parameter("rkA", [128, nblkA * (ABLK // 128)], f32, isOutput=False)
    rkB = nc.declare_dram_parameter("rkB", [128, nblkB * (BBLK // 128)], f32, isOutput=False)
    dsc = nc.declare_dram_parameter("dsc", [128, ndsc16], i16, isOutput=False)
    alphaT = nc.declare_dram_parameter("alphaT", [128, NT], f32, isOutput=False)
    betaT = nc.declare_dram_parameter("betaT", [128, NT], f32, isOutput=False)
    featsh = nc.declare_dram_parameter("featsh", [OUT_ROWS, DIM], f32, isOutput=False)
    WT = nc.declare_dram_parameter("WT", [2, 128, DIM], f32, isOutput=False)
    bias = nc.declare_dram_parameter("bias", [1, DIM], f32, isOutput=False)
    ident = nc.declare_dram_parameter("ident", [128, 128], f32, isOutput=False)
    iota = nc.declare_dram_parameter("iota", [128, ABLK], f32, isOutput=False)

    # pre-zeroed (donated zero buffers)
    msum = nc.declare_dram_parameter("msum", [MSUM_ROWS, DIM], f32, isOutput=True)
    out = nc.declare_dram_parameter("out", [OUT_ROWS, DIM], f32, isOutput=True)

    featA = feature[0:SPLIT]
    featB = feature[SPLIT:N_NODES]

    with tile.TileContext(nc) as tc:
        nc.gpsimd.load_library(mlp)
        with tc.tile_pool(name="const", bufs=1) as cpool:
            siA_sb = cpool.tile([128, nblkA * (ABLK // 16)], i16)
            siB_sb = cpool.tile([128, nblkB * (BBLK // 16)], i16)
            rkA_sb = cpool.tile([128, nblkA * (ABLK // 128)], f32)
            rkB_sb = cpool.tile([128, nblkB * (BBLK // 128)], f32)
            dsc_sb = cpool.tile([128, ndsc16], i16)
            iota_sb = cpool.tile([128, ABLK // 128, 128], f32)
            nc.sync.dma_start(out=siA_sb[:], in_=siA[:])
            nc.sync.dma_start(out=siB_sb[:], in_=siB[:])
            nc.sync.dma_start(out=rkA_sb[:], in_=rkA[:])
            nc.sync.dma_start(out=rkB_sb[:], in_=rkB[:])
            nc.sync.dma_start(out=dsc_sb[:], in_=dsc[:])
            nc.sync.dma_start(out=iota_sb[:], in_=iota.rearrange("p (c m) -> p c m", m=128))

            # ---------------- phase 1: gather + block segment sums ----------
            with (
                tc.tile_pool(name="slots", bufs=3) as spool,
                tc.tile_pool(name="smat", bufs=2) as smpool,
                tc.tile_pool(name="psum", bufs=4, space="PSUM") as ppool,
                tc.tile_pool(name="segsum", bufs=2) as sgpool,
            ):
                def do_block(k, segbatch, tab, si_sb, rk_sb, blk):
                    cols = blk // 128
                    ccols = GCALL // 128
                    slots = spool.tile([128, cols, DIM], f32, tag="slots")
                    for q in range(blk // GCALL):
                        nc.gpsimd.dma_gather(
                            slots[:, q * ccols:(q + 1) * ccols, :], tab,
                            si_sb[:, q * (GCALL // 16):(q + 1) * (GCALL // 16)],
                            GCALL, GCALL, DIM, queue_num=0)
                    smat = smpool.tile([128, cols, 128], f32, tag="smat")
                    nc.vector.tensor_tensor(
                        smat[:], iota_sb[:, 0:cols, :],
                        rk_sb.unsqueeze(-1).broadcast_to([128, cols, 128]),
                        mybir.AluOpType.is_equal)
                    acc = ppool.tile([MAXSEG, DIM], f32, tag="acc")
                    for j in range(cols):
                        nc.tensor.matmul(acc[:], smat[:, j, :], slots[:, j, :],
                                         start=(j == 0), stop=(j == cols - 1))
                    nc.vector.tensor_copy(segbatch[:, k, :], acc[:])

                def do_bucket(dbase16, nblk, grp, tab, si_sb, rk_sb, blk):
                    cols, c16 = blk // 128, blk // 16
                    g16 = grp * MAXSEG // 16
                    for g in range(nblk // grp):
                        segbatch = sgpool.tile([128, GRP, DIM], f32, tag="segb")
                        for k in range(grp):
                            b = g * grp + k
                            do_block(k, segbatch, tab,
                                     si_sb[:, b * c16:(b + 1) * c16],
                                     rk_sb[:, b * cols:(b + 1) * cols], blk)
                        o16 = dbase16 + g * g16
                        nc.gpsimd.dma_scatter_add(
                            msum[:], segbatch[:, 0:grp, :],
                            dsc_sb[:, o16:o16 + g16],
                            grp * MAXSEG, grp * MAXSEG, DIM, queue_num=0)

                do_bucket(0, nblkA, grpA, featA, siA_sb, rkA_sb, ABLK)
                do_bucket(ngrpA * grpA * (MAXSEG // 16), nblkB, grpB,
                          featB, siB_sb, rkB_sb, BBLK)

            # ---------------- phase 2: mean/blend + linear -------------------
            with (
                tc.tile_pool(name="p2", bufs=3) as p2,
                tc.tile_pool(name="p2c", bufs=1) as p2c,
                tc.tile_pool(name="pt", bufs=2, space="PSUM") as pt,
                tc.tile_pool(name="po", bufs=2, space="PSUM") as po,
            ):
                wt_sb = p2c.tile([128, 2, DIM], f32)
                nc.sync.dma_start(out=wt_sb[:], in_=WT.rearrange("k p d -> p k d"))
                bias_sb = p2c.tile([1, DIM], f32)
                nc.sync.dma_start(out=bias_sb[:], in_=bias[:])
                id_sb = p2c.tile([128, 128], f32)
                nc.sync.dma_start(out=id_sb[:], in_=ident[:])
                ones_sb = p2c.tile([1, 128], f32)
                nc.vector.memset(ones_sb[:], 1.0)
                al_sb = p2c.tile([128, NT], f32)
                nc.sync.dma_start(out=al_sb[:], in_=alphaT[:])
                be_sb = p2c.tile([128, NT], f32)
                nc.sync.dma_start(out=be_sb[:], in_=betaT[:])

                for t in range(NT):
                    ms = p2.tile([128, DIM], f32, tag="ms")
                    nc.sync.dma_start(out=ms[:], in_=msum[t * 128:(t + 1) * 128])
                    ft = p2.tile([128, DIM], f32, tag="ft")
                    nc.sync.dma_start(out=ft[:], in_=featsh[t * 128:(t + 1) * 128])
                    h = p2.tile([128, DIM], f32, tag="h")
                    nc.vector.tensor_scalar_mul(h[:], ms[:], al_sb[:, t:t + 1])
                    f2 = p2.tile([128, DIM], f32, tag="f2")
                    nc.vector.tensor_scalar_mul(f2[:], ft[:], be_sb[:, t:t + 1])
                    nc.vector.tensor_add(h[:], h[:], f2[:])
                    hts = []
                    for k in range(2):
                        ptile = pt.tile([128, 128], f32, tag="ptr")
                        nc.tensor.transpose(ptile[:], h[:, k * 128:(k + 1) * 128], id_sb[:])
                        ht = p2.tile([128, 128], f32, tag=f"ht{k}")
                        nc.vector.tensor_copy(ht[:], ptile[:])
                        hts.append(ht)
                    acc = po.tile([128, DIM], f32, tag="oacc")
                    nc.tensor.matmul(acc[:], hts[0][:], wt_sb[:, 0, :], start=True, stop=False)
                    nc.tensor.matmul(acc[:], hts[1][:], wt_sb[:, 1, :], start=False, stop=False)
                    nc.tensor.matmul(acc[:], ones_sb[:], bias_sb[:], start=False, stop=True)
                    ot = p2.tile([128, DIM], f32, tag="ot")
                    nc.vector.tensor_copy(ot[:], acc[:])
                    nc.sync.dma_start(out=out[t * 128:(t + 1) * 128], in_=ot[:])

    nc.compile()
    return nc


def _run_spmd(nc, in_maps, repeats=0):
    """SPMD execute `nc` on the 8 cores (one compile); if repeats > 0, time
    that many extra executions with device-resident inputs. Mirrors
    concourse.bass2jax.run_bass_via_pjrt's sharded path."""
    import jax
    from jax.sharding import Mesh, PartitionSpec
    from jax.experimental.shard_map import shard_map
    from concourse import bass2jax, mybir as mb

    bass2jax.install_neuronx_cc_hook()
    n_cores = len(in_maps)
    partition_name = (nc.partition_id_tensor.name
                      if nc.partition_id_tensor else None)

    in_names, out_names, out_avals, zero_outs = [], [], [], []
    for alloc in nc.m.functions[0].allocations:
        if not isinstance(alloc, mb.MemoryLocationSet):
            continue
        name = alloc.memorylocations[0].name
        if alloc.kind == "ExternalInput":
            if name != partition_name:
                in_names.append(name)
        elif alloc.kind == "ExternalOutput":
            shape = tuple(alloc.tensor_shape)
            dtype = mb.dt.np(alloc.dtype)
            out_names.append(name)
            out_avals.append(jax.core.ShapedArray(shape, dtype))
            zero_outs.append(np.zeros(shape, dtype))
    n_params = len(in_names)
    n_outs = len(out_avals)
    all_in_names = list(in_names) + list(out_names)
    if partition_name is not None:
        all_in_names.append(partition_name)

    donate = tuple(range(n_params, n_params + n_outs))

    def _body(*args):
        operands = list(args)
        if partition_name is not None:
            operands.append(bass2jax.partition_id_tensor())
        outs = bass2jax._bass_exec_p.bind(
            *operands,
            out_avals=tuple(out_avals),
            in_names=tuple(all_in_names),
            out_names=tuple(out_names),
            lowering_input_output_aliases=(),
            sim_require_finite=True,
            sim_require_nnan=True,
            nc=nc,
        )
        return tuple(outs)

    devices = jax.devices()[:n_cores]
    mesh = Mesh(np.asarray(devices), ("core",))
    sharded = jax.jit(
        shard_map(_body, mesh=mesh,
                  in_specs=(PartitionSpec("core"),) * (n_params + n_outs),
                  out_specs=(PartitionSpec("core"),) * n_outs,
                  check_rep=False),
        donate_argnums=donate, keep_unused=True)

    concat_in = [np.concatenate([in_maps[c][name] for c in range(n_cores)], axis=0)
                 for name in in_names]

    def fresh_zeros():
        return [np.zeros((n_cores * z.shape[0], *z.shape[1:]), z.dtype)
                for z in zero_outs]

    out_arrs = jax.block_until_ready(sharded(*concat_in, *fresh_zeros()))
    results = [
        {name: np.asarray(out_arrs[i]).reshape(n_cores, *out_avals[i].shape)[c]
         for i, name in enumerate(out_names)}
        for c in range(n_cores)
    ]

    times = []
    if repeats > 0:
        sharding = jax.sharding.NamedSharding(mesh, PartitionSpec("core"))
        dev_in = [jax.device_put(a, sharding) for a in concat_in]
        zsets = [[jax.device_put(z, sharding) for z in fresh_zeros()]
                 for _ in range(repeats)]
        jax.block_until_ready(dev_in)
        jax.block_until_ready(zsets)
        for r in range(repeats):
            t0 = time.perf_counter()
            jax.block_until_ready(sharded(*dev_in, *zsets[r]))
            times.append(time.perf_counter() - t0)
    return results, times


def kernel(feature, src, dst, W, b, profile=False, repeats=5):
    global _last_exec_ns, _last_times
    feature = np.asarray(feature, dtype=np.float32)
    src = np.asarray(src).astype(np.int64)
    dst = np.asarray(dst).astype(np.int64)
    W = np.asarray(W, dtype=np.float32)
    b = np.asarray(b, dtype=np.float32)

    cores = [_build_core_inputs(c, feature, src, dst, W, b) for c in range(NCORES)]
    nblkA = max(len(c["blocksA"]) for c in cores)
    nblkB = max(len(c["blocksB"]) for c in cores)
    grpA, grpB = _grp_for(nblkA), _grp_for(nblkB)
    nblkA, nblkB = _round_up(nblkA, grpA), _round_up(nblkB, grpB)
    in_maps = [_finalize_core_inputs(c, nblkA, nblkB, grpA, grpB, feature, W, b)
               for c in cores]

    nc = _build_program(nblkA, nblkB, grpA, grpB)
    results, times = _run_spmd(nc, in_maps, repeats=repeats if profile else 0)
    _last_times = times
    _last_exec_ns = int(min(times) * 1e9) if times else None
    out = np.concatenate([results[c]["out"][:NSH] for c in range(NCORES)], axis=0)
    return out
